# revision 4
# baseline (speedup 1.0000x reference)
"""Trainium2 Bass kernel for AttnPainterOil-style top-K stroke compositing.

Problem semantics (per pixel, fully independent):
  draw[n] = (n+1) * (alpha[n] > 0.1); top-K=10 of draw over N=256 strokes
  (descending) == the 10 highest-index strokes with alpha > 0.1.  Gather
  alpha/color at those indices and composite back-to-front over white.

Device formulation (front-to-back, strokes in descending index order):
maintain per-pixel transmittance T (init 1) and a raw pass-count R.  For
stroke s with host-masked alpha ae_s (= a * 1{a > 0.1}, fp32-exact mask
applied on host, shipped as fp16):
  gate m_s = 1{R_s < 10} with R_s = #passing among strokes < s (RAW count,
  independent of gating -- so it batches), ta_s = m_s * b_s * T_quad, where
  b_s are quad-local exclusive-prefix weights b_j = ae_j * prod_{i<j}(1-ae_i)
  (batch-precomputed).  Within a quad the gate mask is a suffix cut, so
  ta_j = m_j * b_j * T is exact and T_new = T - sum_j ta_j.

Key perf facts measured on TRN2 for this kernel family:
  - DVE op = ~150ns dispatch + free_elems cycles @0.96GHz; fp16 with unit
    innermost stride runs 2x.  Broadcasts on outer dims keep 2x.  STT = 1x.
  - Each dma_start instruction costs 600-1300ns of issue time on its queue
    engine; batch transfers and split across the Sync + GpSimd queues.
  - ACT (scalar engine) is otherwise idle: 1-ae, pass bits (Sign), small
    copies at 1 elem/cycle/lane.  Its function-table load is hoisted by the
    compiler to kernel start.
  - fp16 everywhere passes easily (measured 1.0e-3 vs 2e-2 tolerance).
  - Depth 20 suffices for this input (every pixel's 10th passing stroke is
    within the top 20; checked on host, exact numpy fallback otherwise).
  - PE accumulates ta*c into PSUM via fp16 identity matmuls.

Sharding: pure data parallel, one batch element per NeuronCore (B=8).
"""

import numpy as np

B, N, W, K = 8, 256, 128, 10
ALPHA_THRESH = 0.1
D = 20          # strokes processed from the top (host-verified sufficient)
P = 128         # partitions (pixel rows)
F = 128         # free dim (pixel cols)
NCORES = 8

_nc_cache = {}


def _build_nc(depth):
    import concourse.bass as bass  # noqa: F401
    import concourse.tile as tile
    from concourse import bacc, mybir
    from concourse.vector_clock import ScopedClock

    op = mybir.AluOpType
    act = mybir.ActivationFunctionType
    f16 = mybir.dt.float16
    f32 = mybir.dt.float32

    assert depth % 4 == 0
    NQ = depth // 4          # quads (5)
    NG = NQ - 2              # gated quads (2..NQ-1; strokes 8..depth-1)
    SB = depth - 4           # strokes in the second alpha chunk (quads 1..)

    class _OneShotTileContext(tile.TileContext):
        """TileContext with a slim exit: the drain alone (it waits on the
        global clock, including output-DMA completion) -- no all-engine
        barriers and no per-semaphore clears.  Safe because every
        run_bass_kernel_spmd call builds and loads a fresh executable."""

        def _drain_and_barrier(self, tick_clock, wait_clock):
            drain_inst = self.nc.sync.drain()
            wait_clock.add_sem_waits(
                drain_inst.ins, ScopedClock({None: tick_clock.global_clock})
            )
            popped = self.nc._tile_sem_poison_stack.pop()
            assert popped is self._sem_poison

    nc = bacc.Bacc("TRN2", target_bir_lowering=False, debug=False)

    ae_d = nc.dram_tensor("ae_in", [P, depth * F], f16, kind="ExternalInput").ap()
    c_d = nc.dram_tensor("color_in", [P, depth * 3 * F], f16, kind="ExternalInput").ap()
    ident_d = nc.dram_tensor("ident_in", [P, P], f16, kind="ExternalInput").ap()
    out_d = nc.dram_tensor("out", [P, 3 * F], f16, kind="ExternalOutput").ap()

    with _OneShotTileContext(nc) as tc:
        with (
            tc.tile_pool(name="const", bufs=1) as constp,
            tc.tile_pool(name="slab", bufs=1) as slabp,
            tc.tile_pool(name="work", bufs=2) as workp,
            tc.tile_pool(name="prodp", bufs=2) as prodp,
            tc.tile_pool(name="psum", bufs=1, space="PSUM") as psump,
        ):
            # gpsimd: memsets first (cheap), then the color DMAs so their
            # issue cost runs parallel to the Sync queue's alpha DMAs.
            T = constp.tile([P, F], f16)
            R = constp.tile([P, F], f16)
            nc.gpsimd.memset(T[:], 1.0)
            part = slabp.tile([P, NG, 4, F], f16)
            nc.gpsimd.memset(part[:, :, 0], 0.0)

            # ---- input DMAs ----
            # Sync queue: tiny first alpha chunk (quad 0) -> rest -> ident.
            ae0 = slabp.tile([P, 4, F], f16)
            aeB = slabp.tile([P, SB, F], f16)
            nc.sync.dma_start(
                ae0[:], ae_d[:, : 4 * F].rearrange("p (s f) -> p s f", s=4)
            )
            nc.sync.dma_start(
                aeB[:], ae_d[:, 4 * F :].rearrange("p (s f) -> p s f", s=SB)
            )
            ident = constp.tile([P, P], f16)
            nc.sync.dma_start(ident[:], ident_d)
            # GpSimd queue: colors in two big chunks (quads 0-2, quads 3-4).
            ctile = slabp.tile([P, depth, 3, F], f16)
            nc.gpsimd.dma_start(
                ctile[:, :12],
                c_d[:, : 12 * 3 * F].rearrange("p (s c f) -> p s c f", s=12, c=3),
            )
            nc.gpsimd.dma_start(
                ctile[:, 12:],
                c_d[:, 12 * 3 * F :].rearrange(
                    "p (s c f) -> p s c f", s=depth - 12, c=3
                ),
            )

            # ---- q = 1 - ae ----
            # quad 0 on DVE (unblocks the chain fast): q0 = (ae*-1) + 1
            q0 = slabp.tile([P, 4, F], f16)
            nc.vector.tensor_scalar(
                q0[:], ae0[:], -1.0, 1.0, op0=op.mult, op1=op.add
            )
            # rest on ACT, split so quad 1 unblocks early
            qB = slabp.tile([P, SB, F], f16)
            nc.scalar.activation(qB[:, :4], aeB[:, :4], act.Relu, bias=1.0, scale=-1.0)
            nc.scalar.activation(qB[:, 4:], aeB[:, 4:], act.Relu, bias=1.0, scale=-1.0)
            # pass bits (strokes 0..depth-2) on ACT
            passA = slabp.tile([P, 4, F], f16)          # strokes 0..3
            passB = slabp.tile([P, SB - 1, F], f16)     # strokes 4..depth-2
            nc.scalar.sign(passA[:], ae0[:])
            nc.scalar.sign(passB[:], aeB[:, : SB - 1])

            # ---- batched b-term precompute (quad-local exclusive prefixes)
            # positions within quad: 0: ae; 1: ae*q0; 2: ae*q01; 3: ae*q012
            def build_b(aet, qt, nq, btile, qq, nqq):
                aeQ = aet[:].rearrange("p (qd s) f -> p qd s f", s=4)
                qQ = qt[:].rearrange("p (qd s) f -> p qd s f", s=4)
                bQ = btile[:].rearrange("p (qd s) f -> p qd s f", s=4)
                aeP = aet[:].rearrange("p (pr two) f -> p pr two f", two=2)
                qP = qt[:].rearrange("p (pr two) f -> p pr two f", two=2)
                # pos 0 copy on ACT (frees DVE)
                nc.scalar.copy(bQ[:, :, 0], aeQ[:, :, 0])
                # q12 per pair
                q12 = workp.tile([P, 2 * nq, F], f16, tag="q12")
                nc.vector.tensor_tensor(q12[:], qP[:, :, 0], qP[:, :, 1], op=op.mult)
                q12P = q12[:].rearrange("p (qd two) f -> p qd two f", two=2)
                # pos 1 = ae1*q0
                nc.vector.tensor_tensor(
                    bQ[:, :, 1], aeQ[:, :, 1], qQ[:, :, 0], op=op.mult
                )
                # pos 2 = ae2*q12
                nc.vector.tensor_tensor(
                    bQ[:, :, 2], aeQ[:, :, 2], q12P[:, :, 0], op=op.mult
                )
                # pos 3 = (ae3*q2)*q12
                t4 = workp.tile([P, nq, F], f16, tag="t4")
                nc.vector.tensor_tensor(t4[:], aeQ[:, :, 3], qQ[:, :, 2], op=op.mult)
                nc.vector.tensor_tensor(
                    bQ[:, :, 3], t4[:], q12P[:, :, 0], op=op.mult
                )
                if nqq:   # quad products for ungated T-updates (first nqq quads)
                    nc.vector.tensor_tensor(
                        qq[:], q12P[:, :nqq, 0], q12P[:, :nqq, 1], op=op.mult
                    )
                return bQ

            b0 = slabp.tile([P, 4, F], f16)
            qq0 = slabp.tile([P, 1, F], f16)
            bQ0 = build_b(ae0, q0, 1, b0, qq0, 1)

            bB = slabp.tile([P, SB, F], f16)
            qqB = slabp.tile([P, 1, F], f16)
            bQB = build_b(aeB, qB, NQ - 1, bB, qqB, 1)

            # ---- gate machinery (batched): pair/quad sums of pass bits,
            # intra-quad partial prefixes for gated quads.
            psA = slabp.tile([P, 2, F], f16)
            pAP = passA[:].rearrange("p (pr two) f -> p pr two f", two=2)
            nc.vector.tensor_tensor(psA[:], pAP[:, :, 0], pAP[:, :, 1], op=op.add)
            qsA = slabp.tile([P, F], f16)
            nc.vector.tensor_tensor(qsA[:], psA[:, 0], psA[:, 1], op=op.add)

            npB = (SB - 2) // 2                          # pairs in chunk B
            psB = slabp.tile([P, npB, F], f16)
            pBP = passB[:, : 2 * npB].rearrange("p (pr two) f -> p pr two f", two=2)
            nc.vector.tensor_tensor(psB[:], pBP[:, :, 0], pBP[:, :, 1], op=op.add)
            qsB = slabp.tile([P, NG, F], f16)            # quads 1..NQ-2
            nc.vector.tensor_tensor(
                qsB[:], psB[:, 0 : 2 * NG : 2], psB[:, 1 : 2 * NG : 2], op=op.add
            )

            # partials for gated quads: j=0: 0; j=1: p0; j=2: p0+p1; j=3: p0+p1+p2
            nc.scalar.copy(part[:, :, 1], passB[:, 4::4])     # strokes 8,12,16
            nc.scalar.copy(part[:, :, 2], psB[:, 2::2])       # pairs 4,6,8
            nc.vector.tensor_tensor(
                part[:, :, 3], psB[:, 2::2], passB[:, 6::4], op=op.add
            )

            # ---- serial chain over quads ----
            cacc = psump.tile([P, 3 * F], f32)
            for qi in range(NQ):
                gated = qi >= 2
                if qi == 0:
                    b_quad = bQ0[:, 0]
                else:
                    b_quad = bQB[:, qi - 1]
                T_b = T[:].unsqueeze(1).broadcast_to((P, 4, F))
                ta = workp.tile([P, 4, F], f16, tag="ta")
                if not gated:
                    nc.vector.tensor_tensor(ta[:], b_quad, T_b, op=op.mult)
                else:
                    tmp = workp.tile([P, 4, F], f16, tag="tmp")
                    R_b = R[:].unsqueeze(1).broadcast_to((P, 4, F))
                    nc.vector.tensor_tensor(tmp[:], part[:, qi - 2], R_b, op=op.add)
                    mb = workp.tile([P, 4, F], f16, tag="mb")
                    nc.vector.scalar_tensor_tensor(
                        mb[:], tmp[:], float(K) - 0.5, b_quad,
                        op0=op.is_lt, op1=op.mult,
                    )
                    nc.vector.tensor_tensor(ta[:], mb[:], T_b, op=op.mult)

                prod = prodp.tile([P, 4, 3, F], f16, tag="prod")
                ta_b = ta[:].unsqueeze(2).broadcast_to((P, 4, 3, F))
                nc.vector.tensor_tensor(
                    prod[:], ctile[:, 4 * qi : 4 * qi + 4], ta_b, op=op.mult
                )
                for j in range(4):
                    s = 4 * qi + j
                    nc.tensor.matmul(
                        cacc[:], ident[:],
                        prod[:, j].rearrange("p c f -> p (c f)"),
                        start=(s == 0), stop=(s == depth - 1),
                    )

                # T update (after prods consumed ta)
                if not gated:
                    qq = qq0 if qi == 0 else qqB
                    nc.vector.tensor_tensor(T[:], T[:], qq[:, 0], op=op.mult)
                else:
                    h = workp.tile([P, 2, F], f16, tag="h")
                    nc.vector.tensor_tensor(
                        h[:], ta[:, 0:2], ta[:, 2:4], op=op.add
                    )
                    nc.vector.tensor_tensor(T[:], T[:], h[:, 0], op=op.subtract)
                    nc.vector.tensor_tensor(T[:], T[:], h[:, 1], op=op.subtract)

                # R update (raw pass count at next quad start)
                if qi == 1:
                    nc.vector.tensor_tensor(R[:], qsA[:], qsB[:, 0], op=op.add)
                elif gated and qi < NQ - 1:
                    nc.vector.tensor_tensor(R[:], R[:], qsB[:, qi - 1], op=op.add)

            # out = cacc + T (white background), straight out of PSUM; split
            # in two halves so the first DMA issue overlaps the second add.
            outt = constp.tile([P, 3, F], f16)
            caccv = cacc[:].rearrange("p (c f) -> p c f", c=3)
            T_b3 = T[:].unsqueeze(1)
            nc.vector.tensor_tensor(
                outt[:, 0:2], caccv[:, 0:2],
                T_b3.broadcast_to((P, 2, F)), op=op.add,
            )
            nc.sync.dma_start(
                out_d[:, : 2 * F], outt[:, 0:2].rearrange("p c f -> p (c f)")
            )
            nc.vector.tensor_tensor(
                outt[:, 2:3], caccv[:, 2:3],
                T_b3.broadcast_to((P, 1, F)), op=op.add,
            )
            nc.sync.dma_start(
                out_d[:, 2 * F :], outt[:, 2:3].rearrange("p c f -> p (c f)")
            )

    nc.compile()
    return nc


def _prep_inputs(color_stroke, alpha, depth):
    """Slice the top `depth` strokes (reversed so stroke 0 = highest index),
    mask alpha by the fp32-exact threshold on host, convert to fp16 and lay
    out per core: ae [P, depth*F], color [P, depth*3*F]."""
    a_r = alpha[:, N - depth :, 0][:, ::-1]          # (B, depth, P, F)
    ae_host = (a_r * (a_r > ALPHA_THRESH)).astype(np.float16)
    c_r = color_stroke[:, N - depth :][:, ::-1].astype(np.float16)  # (B,depth,3,P,F)
    ident = np.eye(P, dtype=np.float16)
    in_maps = []
    for b in range(B):
        ae_core = np.ascontiguousarray(ae_host[b].transpose(1, 0, 2)).reshape(
            P, depth * F
        )
        c_core = np.ascontiguousarray(c_r[b].transpose(2, 0, 1, 3)).reshape(
            P, depth * 3 * F
        )
        in_maps.append({"ae_in": ae_core, "color_in": c_core, "ident_in": ident})
    return in_maps


def _reference_numpy(color_stroke, alpha):
    """Exact replication of the oracle (incl. top-k tie-breaking) on host.
    Only used when the depth-cutoff precondition fails (pathological inputs)."""
    stroke_ids = np.arange(1, N + 1, dtype=np.int32).reshape(1, N, 1, 1)
    draw = stroke_ids * (alpha[:, :, 0] > ALPHA_THRESH).astype(np.int32)  # (B,N,W,W)
    draw_t = np.moveaxis(draw, 1, -1)  # (B,W,W,N)
    idx = np.argsort(-draw_t, axis=-1, kind="stable")[..., :K]  # (B,W,W,K)
    idx = np.moveaxis(idx, -1, 1)[:, :, None]  # (B,K,1,W,W)
    alpha_k = np.take_along_axis(alpha, idx, axis=1)  # (B,K,1,W,W)
    color_k = np.take_along_axis(color_stroke, idx, axis=1)  # (B,K,3,W,W)
    canvas = np.ones((B, 3, W, W), dtype=color_stroke.dtype)
    for i in range(K - 1, -1, -1):
        a = alpha_k[:, i]
        canvas = canvas * (1.0 - a) + a * color_k[:, i]
    return canvas


def kernel(color_stroke, alpha):
    color_stroke = np.asarray(color_stroke, dtype=np.float32)
    alpha = np.asarray(alpha, dtype=np.float32)
    assert color_stroke.shape == (B, N, 3, W, W), color_stroke.shape
    assert alpha.shape == (B, N, 1, W, W), alpha.shape

    # Precondition for the depth cutoff: every pixel finds its 10 passing
    # strokes within the top D.
    top_pass = (alpha[:, N - D :, 0] > ALPHA_THRESH).sum(axis=1)
    if top_pass.min() < K:
        return _reference_numpy(color_stroke, alpha)

    from concourse.bass_utils import run_bass_kernel_spmd

    if D not in _nc_cache:
        _nc_cache[D] = _build_nc(D)
    nc = _nc_cache[D]

    in_maps = _prep_inputs(color_stroke, alpha, D)
    res = run_bass_kernel_spmd(nc, in_maps, core_ids=list(range(NCORES)))

    out = np.empty((B, 3, W, W), dtype=np.float32)
    for b in range(B):
        out[b] = (
            res.results[b]["out"].reshape(P, 3, F).transpose(1, 0, 2)
            .astype(np.float32)
        )
    return out
